# revision 5
# baseline (speedup 1.0000x reference)
"""Trainium2 Bass kernel for the attention-based encoder.

Computation (per batch b):
    a      = P @ y[b]                                  # [D]
    logits = x[b] @ a                                  # [M]
    p_un   = exp(logits - 16)                          # fixed shift (cancels)
    Z      = sum(p_un)
    W[t]   = p_un[t-1] + p_un[t] + p_un[t+1] + p_un[t+2]  (zero-padded), W[M-1] = 0
    enc[b] = (W @ x[b]) / (Q * Z)                      # [D]

Design (each point validated by HW ablation / CoreSim timeline analysis):
  * all HBM traffic in bf16 (x 16 MiB + P 10 MiB per core; rel err 3.6e-3
    vs the 2e-2 gate) - halves the DMA floor and runs every matmul at
    bf16 rate instead of multi-pass fp32.
  * host pre-arranges x and P^T so every DMA is one large transfer with
    contiguous 16-80 KiB per partition, and pads the per-partition row
    by 2176 B: the power-of-2 HBM stride otherwise aliases banks
    (187 -> 315 GB/s measured).
  * x loaded as two half-batch tiles (x pool bufs=3, P chunk pool bufs=3:
    with fewer buffers the Tile scheduler reorders x DMAs ahead of the
    last P chunk and delays `a` by ~20 us).
  * logits: per 4-tile group, one fused DVE mul (broadcast a) + one
    strided DVE reduce over the groups' second halves + four ACT
    512-halved accumulates (ACT-1024 accum and DVE-1024 reduce are both
    ~1.3-1.4 us; 512-splitting + fusion balances DVE and ACT).
  * W = 4-tap window of p via 5 banded matmuls per batch over all tiles
    at once (halo via shifted rhs columns of a zero-padded p).
  * a replicated across partitions with a selector matmul; Z via DVE
    reduce + ones-column matmul; single batched output DMA.
  * repeated-execution (reps>1) builds unroll the loop body by 2: the
    plain hardware-loop edge serializes iterations, while a 2-body block
    lets the next rep's P/x DMAs run under this rep's batch compute
    (136 -> 124 us per rep measured; unroll=4 schedules worse).
  * pshard=True phase A (P split over cores + AllReduce) works and is
    equally accurate, but collectives fail inside a reps-loop in this
    runtime, so it is off by default.

Sharding: data-parallel over batch, 4 batches per core on 8 cores.
"""

import numpy as np
import ml_dtypes

import concourse.bass as bass
import concourse.mybir as mybir
from concourse.tile import TileContext
from concourse.bass_utils import run_bass_kernel_spmd

# ---------------------------------------------------------------------------
# This container's walrus supports only ONE sync wait per instruction ("Too
# many sync wait commands" at codegen otherwise), while Tile freely attaches
# several.  Post-pass: hoist excess waits onto injected same-engine NoOps
# placed immediately before the over-subscribed instruction.
# ---------------------------------------------------------------------------

_MAX_WAITS = 1


def split_sync_waits(nc: bass.Bass) -> None:
    uid = 0
    for fn in nc.m.functions:
        for blk in fn.blocks:
            new_insts = []
            for inst in blk.instructions:
                si = inst.sync_info
                waits = list(si.on_wait) if si and si.on_wait else []
                if len(waits) > _MAX_WAITS:
                    for w in waits[:-_MAX_WAITS]:
                        uid += 1
                        ev = mybir.InstEventSemaphore(
                            name=f"{inst.name}_hw{uid}",
                            opcode="EventSemaphore",
                            ins=[],
                            outs=[],
                            sync_info=mybir.SyncInfo(on_wait=[w], on_update=[]),
                        )
                        ev.engine = inst.engine
                        new_insts.append(ev)
                    si.on_wait = waits[-_MAX_WAITS:]
                new_insts.append(inst)
            blk.instructions[:] = new_insts

# ---------------------------------------------------------------------------

B, M, D, CD = 32, 2048, 1024, 5120
Q = 2
NCORES = 8
BPC = B // NCORES          # batches per core
NT = M // 128              # m-tiles per batch
KT = CD // 128             # k-tiles of the P contraction
KCH = 8                    # k-tiles per P DMA chunk
NCH = KT // KCH
KTC = KT // NCORES  # k-tiles per core when P is sharded
PAD = 1088                 # bf16 elems of per-partition-row padding: breaks
                           # the power-of-2 HBM stride aliasing (187 -> 315 GB/s)
HNT = NT // 2              # m-tiles per xb half-load
F32 = mybir.dt.float32
BF16 = mybir.dt.bfloat16
ALU = mybir.AluOpType
AFT = mybir.ActivationFunctionType


def build_nc(reps: int = 1, n_batches: int = BPC, do_phase_b: bool = True,
             stop_after: str = "full", skip: tuple = (),
             logits_mode: str = "split", xhalves: bool = False,
             n_dve: int = 16, n_gps: int = 0,
             accum_dump: str = "inplace", fuse_g: int = 0,
             n_actg: int = 0, pshard: bool = False,
             unroll: int = 2) -> bass.Bass:
    nc = bass.Bass(num_devices=NCORES) if pshard else bass.Bass()
    xs = nc.declare_dram_parameter("xs", [128, BPC, NT * D + PAD], BF16, isOutput=False)
    if pshard:
        # P sharded over cores: each core holds KTC k-tiles of P^T, the y
        # slices matching those k-tiles for ALL B batches, and a selector
        # that picks this core's batch rows out of the allreduced aT.
        pt = nc.declare_dram_parameter("pt", [128, KTC * D + PAD], BF16,
                                       isOutput=False)
        ys = nc.declare_dram_parameter("ys", [128, KTC, B], BF16,
                                       isOutput=False)
        sel = nc.declare_dram_parameter("sel", [B, BPC * 128], BF16,
                                        isOutput=False)
        cc_in = nc.dram_tensor("cc_in", [B, D], F32, addr_space="Local")
        cc_out = nc.dram_tensor("cc_out", [B, D], F32, addr_space="Local")
    else:
        pt = nc.declare_dram_parameter("pt", [128, KT * D + PAD], BF16,
                                       isOutput=False)
        ys = nc.declare_dram_parameter("ys", [128, KT, BPC], BF16,
                                       isOutput=False)
    enc = nc.declare_dram_parameter("enc", [1, BPC * D], F32, isOutput=True)

    with TileContext(nc) as tc:
        with (
            tc.tile_pool(name="const", bufs=1) as const_pool,
            tc.tile_pool(name="ysp", bufs=1) as ys_pool,
            tc.tile_pool(name="ptp", bufs=3) as pt_pool,
            tc.tile_pool(name="xp", bufs=3) as x_pool,
            tc.tile_pool(name="arep", bufs=1) as arep_pool,
            tc.tile_pool(name="small", bufs=1) as small_pool,
            tc.tile_pool(name="tiny", bufs=2) as tiny_pool,
            tc.tile_pool(name="scr", bufs=3) as scr_pool,
            tc.tile_pool(name="ps", bufs=1, space="PSUM") as psum_pool,
            tc.tile_pool(name="pse", bufs=2, space="PSUM") as psum_e_pool,
        ):
            ones_col = const_pool.tile([128, 1], F32)
            nc.vector.memset(ones_col[:], 1.0)
            nshift = const_pool.tile([128, 1], F32)
            nc.vector.memset(nshift[:], -16.0)
            if pshard:
                ys_sb = const_pool.tile([128, KTC, B], BF16)
                nc.sync.dma_start(out=ys_sb[:], in_=ys[:])
                sel_sb = const_pool.tile([B, BPC * 128], BF16)
                nc.sync.dma_start(out=sel_sb[:], in_=sel[:])
            else:
                ys_sb = const_pool.tile([128, KT, BPC], BF16)
                nc.sync.dma_start(out=ys_sb[:], in_=ys[:])

            # banded matrices for the 4-tap sliding-window sum W = S4 @ p:
            # s4[c, f] = 1 iff f - c in {-2, -1, 0, 1}; corner matrices
            # carry the +-2-element inter-tile halo.
            s4 = const_pool.tile([128, 128], F32)
            nc.gpsimd.memset(s4[:], 0.0)
            for base in (1, 0, -1, -2):
                nc.gpsimd.affine_select(
                    out=s4[:], in_=s4[:], compare_op=ALU.not_equal, fill=1.0,
                    base=base, pattern=[[-1, 128]], channel_multiplier=1,
                )
            sprev = const_pool.tile([128, 128], F32)
            nc.gpsimd.memset(sprev[:], 0.0)
            nc.gpsimd.affine_select(
                out=sprev[:], in_=sprev[:], compare_op=ALU.not_equal, fill=1.0,
                base=-127, pattern=[[-1, 128]], channel_multiplier=1,
            )
            snext = const_pool.tile([128, 128], F32)
            nc.gpsimd.memset(snext[:], 0.0)
            for base in (126, 127):
                nc.gpsimd.affine_select(
                    out=snext[:], in_=snext[:], compare_op=ALU.not_equal, fill=1.0,
                    base=base, pattern=[[-1, 128]], channel_multiplier=1,
                )

            # last-tile variant of s4 with column M-1 zeroed (W[M-1] = 0)
            s4last = const_pool.tile([128, 128], F32)
            nc.gpsimd.memset(s4last[:], 0.0)
            for base in (1, 0, -1, -2):
                nc.gpsimd.affine_select(
                    out=s4last[:], in_=s4last[:], compare_op=ALU.not_equal,
                    fill=1.0, base=base, pattern=[[-1, 128]],
                    channel_multiplier=1,
                )
            nc.gpsimd.affine_select(
                out=s4last[:], in_=s4last[:], compare_op=ALU.not_equal,
                fill=0.0, base=-127, pattern=[[1, 128]], channel_multiplier=0,
            )

            # selector rows: selb[b][k, m] = 1 iff k == b (4 partitions);
            # a_rep[b] = selb[b].T @ aT broadcasts batch b's row of aT to
            # all 128 partitions without an SBUF round-trip.
            selb = []
            for b in range(BPC):
                sb = const_pool.tile([BPC, 128], BF16, name=f"selb{b}")
                nc.gpsimd.memset(sb[:], 0.0)
                nc.gpsimd.affine_select(
                    out=sb[:], in_=sb[:], compare_op=ALU.not_equal, fill=1.0,
                    base=-b, pattern=[[0, 128]], channel_multiplier=1,
                )
                selb.append(sb)

            a_rep = [
                arep_pool.tile([128, D], BF16, tag=f"a_rep{b}", name=f"a_rep{b}")
                for b in range(BPC)
            ]

            def body(_=None):
                if "phase_a" in skip:
                    [nc.vector.memset(ar[:], 0.001) for ar in a_rep]
                    return body_b()

                if pshard:
                    return phase_a_sharded()

                # ---- Phase A: aT[b, d] = sum_k y[b, k] * PT[k, d] ----
                pa0 = psum_pool.tile([BPC, 512], F32, tag="pa0")
                pa1 = psum_pool.tile([BPC, 512], F32, tag="pa1")
                for c in range(NCH):
                    ptc = pt_pool.tile([128, KCH * D], BF16, tag="ptc")
                    nc.sync.dma_start(
                        out=ptc[:], in_=pt[:, c * KCH * D:(c + 1) * KCH * D]
                    )
                    for u in range(KCH):
                        t = c * KCH + u
                        nc.tensor.matmul(
                            pa0[:], lhsT=ys_sb[:, t, :],
                            rhs=ptc[:, u * D:u * D + 512],
                            start=(t == 0), stop=(t == KT - 1),
                        )
                        nc.tensor.matmul(
                            pa1[:], lhsT=ys_sb[:, t, :],
                            rhs=ptc[:, u * D + 512:(u + 1) * D],
                            start=(t == 0), stop=(t == KT - 1),
                        )
                aT_sb = small_pool.tile([BPC, D], BF16, tag="aT")
                nc.vector.tensor_copy(aT_sb[:, 0:512], pa0[:])
                nc.vector.tensor_copy(aT_sb[:, 512:1024], pa1[:])

                # replicate a[b] across all 128 partitions (selector matmul)
                for b in range(BPC):
                    for dh in range(2):
                        pr = psum_pool.tile([128, 512], F32, tag="pr")
                        nc.tensor.matmul(
                            pr[:], lhsT=selb[b][:],
                            rhs=aT_sb[:, dh * 512:(dh + 1) * 512],
                            start=True, stop=True,
                        )
                        nc.vector.tensor_copy(
                            a_rep[b][:, dh * 512:(dh + 1) * 512], pr[:]
                        )

                if not do_phase_b:
                    for b in range(BPC):
                        nc.gpsimd.dma_start(
                            out=enc[0, b * D:b * D + 512],
                            in_=a_rep[b][0:1, 0:512],
                        )
                    return
                return body_b()

            def phase_a_sharded():
                # partial aT for ALL batches from this core's P k-tiles,
                # then an 8-core AllReduce combines the partials.
                pa0 = psum_pool.tile([B, 512], F32, tag="pa0")
                pa1 = psum_pool.tile([B, 512], F32, tag="pa1")
                ptc = pt_pool.tile([128, KTC * D], BF16, tag="ptc")
                nc.sync.dma_start(out=ptc[:], in_=pt[:, 0:KTC * D])
                for kt in range(KTC):
                    nc.tensor.matmul(
                        pa0[:], lhsT=ys_sb[:, kt, :],
                        rhs=ptc[:, kt * D:kt * D + 512],
                        start=(kt == 0), stop=(kt == KTC - 1),
                    )
                    nc.tensor.matmul(
                        pa1[:], lhsT=ys_sb[:, kt, :],
                        rhs=ptc[:, kt * D + 512:(kt + 1) * D],
                        start=(kt == 0), stop=(kt == KTC - 1),
                    )
                aT_part = small_pool.tile([B, D], F32, tag="aT_part")
                nc.vector.tensor_copy(aT_part[:, 0:512], pa0[:])
                nc.vector.tensor_copy(aT_part[:, 512:1024], pa1[:])
                nc.sync.dma_start(out=cc_in[:], in_=aT_part[:])
                nc.gpsimd.collective_compute(
                    "AllReduce", ALU.add,
                    replica_groups=[list(range(NCORES))],
                    ins=[cc_in[:]], outs=[cc_out[:]],
                )
                aT_f32 = small_pool.tile([B, D], F32, tag="aT_f32")
                nc.sync.dma_start(out=aT_f32[:], in_=cc_out[:])
                aT_sb = small_pool.tile([B, D], BF16, tag="aT")
                nc.vector.tensor_copy(aT_sb[:], aT_f32[:])

                for b in range(BPC):
                    for dh in range(2):
                        pr = psum_pool.tile([128, 512], F32, tag="pr")
                        nc.tensor.matmul(
                            pr[:], lhsT=sel_sb[:, b * 128:(b + 1) * 128],
                            rhs=aT_sb[:, dh * 512:(dh + 1) * 512],
                            start=True, stop=True,
                        )
                        nc.vector.tensor_copy(
                            a_rep[b][:, dh * 512:(dh + 1) * 512], pr[:]
                        )
                if not do_phase_b:
                    for b in range(BPC):
                        nc.gpsimd.dma_start(
                            out=enc[0, b * D:b * D + 512],
                            in_=a_rep[b][0:1, 0:512],
                        )
                    return
                return body_b()

            def body_b():
                # ---- Phase B: per-batch attention ----
                # n_dve: reduce-halves handed to DVE tensor_reduce instead of
                # ACT accumulate; n_gps: muls offloaded to GpSimd.
                for b in range(n_batches):
                    if xhalves:
                        # two half-batch loads so compute starts ~6.5us earlier
                        xh = []
                        for h in range(2):
                            xt_ = x_pool.tile([128, HNT * D], BF16, tag=f"xh{h}")
                            nc.sync.dma_start(
                                out=xt_[:],
                                in_=xs[:, b, h * HNT * D:(h + 1) * HNT * D],
                            )
                            xh.append(xt_)
                    else:
                        xb = x_pool.tile([128, NT * D], BF16, tag="xh0")
                        nc.sync.dma_start(out=xb[:], in_=xs[:, b, 0:NT * D])
                        xh = [xb[:, 0:HNT * D], xb[:, HNT * D:NT * D]]

                    # logits[m] = x[m, :] . a  - DVE mul per m-tile, then the
                    # free-dim reduce as two 512-halves on ACT (420ns each vs
                    # 1439ns for a 1024 accum / 1266ns for a DVE reduce).
                    logits_a = tiny_pool.tile([128, NT], F32, tag="logits_a")
                    logits_b = tiny_pool.tile([128, NT], F32, tag="logits_b")
                    if "logits" in skip:
                        nc.vector.memset(logits_a[:], 0.005)
                        nc.vector.memset(logits_b[:], 0.005)
                    elif fuse_g:
                        # G-tile fused DVE ops: one mul + one strided reduce
                        # per group amortizes the ~160-cycle DVE op overhead.
                        G = fuse_g
                        for g in range(NT // G):
                            t0 = g * G
                            h = t0 // HNT
                            xt = xh[h][:, (t0 % HNT) * D:(t0 % HNT + G) * D]
                            scratch = scr_pool.tile([128, G * D], BF16,
                                                    tag="scratch")
                            nc.vector.tensor_mul(
                                scratch[:].rearrange("p (g d) -> p g d", g=G),
                                xt.rearrange("p (g d) -> p g d", g=G),
                                a_rep[b][:, None, :].broadcast_to([128, G, D]),
                            )
                            if g < n_actg:
                                # ACT takes this group's second halves too
                                for u in range(G):
                                    nc.scalar.activation(
                                        out=scratch[:, u * D + 512:(u + 1) * D],
                                        in_=scratch[:, u * D + 512:(u + 1) * D],
                                        func=AFT.Copy,
                                        accum_out=logits_b[:, t0 + u:t0 + u + 1],
                                    )
                            else:
                                sv = scratch[:].rearrange(
                                    "p (g two h) -> p g two h", g=G, two=2)
                                nc.vector.tensor_reduce(
                                    out=logits_b[:, t0:t0 + G],
                                    in_=sv[:, :, 1, :],
                                    axis=mybir.AxisListType.X, op=ALU.add,
                                )
                            for u in range(G):
                                nc.scalar.activation(
                                    out=scratch[:, u * D:u * D + 512],
                                    in_=scratch[:, u * D:u * D + 512],
                                    func=AFT.Copy,
                                    accum_out=logits_a[:, t0 + u:t0 + u + 1],
                                )
                    else:
                        for t in range(NT):
                            xt = xh[t // HNT][:, (t % HNT) * D:(t % HNT + 1) * D]
                            scratch = scr_pool.tile([128, D], BF16, tag="scratch")
                            mul_eng = nc.gpsimd if t < n_gps else nc.vector
                            mul_eng.tensor_mul(scratch[:], xt, a_rep[b][:])
                            if t < n_dve - NT:
                                nc.vector.tensor_reduce(
                                    out=logits_a[:, t:t + 1],
                                    in_=scratch[:, 0:512],
                                    axis=mybir.AxisListType.X, op=ALU.add,
                                )
                            else:
                                if accum_dump == "psum":
                                    dmp = psum_pool.tile([128, 512], F32,
                                                         tag="pa0")
                                    outa = dmp[:]
                                elif accum_dump == "sbuf":
                                    dmp = scr_pool.tile([128, 512], BF16,
                                                        tag="dump")
                                    outa = dmp[:]
                                else:
                                    outa = scratch[:, 0:512]
                                nc.scalar.activation(
                                    out=outa, in_=scratch[:, 0:512],
                                    func=AFT.Copy, accum_out=logits_a[:, t:t + 1],
                                )
                            if t < n_dve:
                                nc.vector.tensor_reduce(
                                    out=logits_b[:, t:t + 1],
                                    in_=scratch[:, 512:1024],
                                    axis=mybir.AxisListType.X, op=ALU.add,
                                )
                            else:
                                if accum_dump == "psum":
                                    dmp = psum_pool.tile([128, 512], F32,
                                                         tag="pa1")
                                    outb = dmp[:]
                                elif accum_dump == "sbuf":
                                    dmp = scr_pool.tile([128, 512], BF16,
                                                        tag="dump")
                                    outb = dmp[:]
                                else:
                                    outb = scratch[:, 512:1024]
                                nc.scalar.activation(
                                    out=outb, in_=scratch[:, 512:1024],
                                    func=AFT.Copy,
                                    accum_out=logits_b[:, t:t + 1],
                                )
                    nc.vector.tensor_add(logits_a[:], logits_a[:], logits_b[:])

                    if stop_after == "logits":
                        nc.sync.dma_start(out=enc[0, b * D:b * D + NT], in_=logits_a[0:1, :])
                        continue

                    if "softmax" in skip:
                        zsum = tiny_pool.tile([1, 1], F32, tag="zsum")
                        nc.vector.memset(zsum[:], 1.0)
                        w_pm = tiny_pool.tile([128, NT], BF16, tag="w_pm")
                        nc.vector.memset(w_pm[:], 0.01)
                        do_tail(b, xh, w_pm, zsum)
                        continue

                    # softmax without the row max: fixed shift (cancels in
                    # enc = sum(W x)/(Q Z)); exp on ACT in [128, NT] space.
                    p_pad = tiny_pool.tile([128, NT + 2], F32, tag="p_pad")
                    zcol = tiny_pool.tile([128, 1], F32, tag="zcol")
                    nc.vector.memset(p_pad[:, 0:1], 0.0)
                    nc.vector.memset(p_pad[:, NT + 1:NT + 2], 0.0)
                    nc.scalar.activation(
                        out=p_pad[:, 1:NT + 1],
                        in_=logits_a[:],
                        func=AFT.Exp,
                        bias=nshift[:],
                        scale=1.0,
                    )
                    nc.vector.tensor_reduce(
                        out=zcol[:], in_=p_pad[:, 1:NT + 1],
                        axis=mybir.AxisListType.X, op=ALU.add,
                    )

                    # Z = sum over partitions of zcol (ones-column matmul)
                    z_ps = psum_pool.tile([1, 1], F32, tag="pr")
                    nc.tensor.matmul(z_ps[:], lhsT=zcol[:], rhs=ones_col[:],
                                     start=True, stop=True)
                    zsum = tiny_pool.tile([1, 1], F32, tag="zsum")
                    nc.scalar.copy(out=zsum[:], in_=z_ps[:])

                    # W = 4-tap window of p: banded matmuls over tiles
                    # 0..14 at once (halo via shifted rhs columns of p_pad);
                    # the last tile separately with s4last (W[M-1] = 0).
                    w_ps = psum_pool.tile([128, NT], F32, tag="w_ps")
                    nc.tensor.matmul(w_ps[:, 0:NT - 1], lhsT=s4[:],
                                     rhs=p_pad[:, 1:NT], start=True, stop=False)
                    nc.tensor.matmul(w_ps[:, 0:NT - 1], lhsT=sprev[:],
                                     rhs=p_pad[:, 0:NT - 1], start=False,
                                     stop=False)
                    nc.tensor.matmul(w_ps[:, 0:NT - 1], lhsT=snext[:],
                                     rhs=p_pad[:, 2:NT + 1], start=False,
                                     stop=True)
                    nc.tensor.matmul(w_ps[:, NT - 1:NT], lhsT=s4last[:],
                                     rhs=p_pad[:, NT:NT + 1], start=True,
                                     stop=False)
                    nc.tensor.matmul(w_ps[:, NT - 1:NT], lhsT=sprev[:],
                                     rhs=p_pad[:, NT - 1:NT], start=False,
                                     stop=True)
                    w_pm = tiny_pool.tile([128, NT], BF16, tag="w_pm")
                    nc.scalar.copy(out=w_pm[:], in_=w_ps[:])

                    do_tail(b, xh, w_pm, zsum)

            def do_tail(b, xh, w_pm, zsum):
                # enc_un[d] = sum_m W[m] x[m, d]   (PE, W cols as weights)
                pe0 = psum_e_pool.tile([1, 512], F32, tag="pe0")
                pe1 = psum_e_pool.tile([1, 512], F32, tag="pe1")
                for t in range(NT):
                    xt = xh[t // HNT][:, (t % HNT) * D:(t % HNT + 1) * D]
                    for dh, pe in enumerate((pe0, pe1)):
                        nc.tensor.matmul(
                            pe[:],
                            lhsT=w_pm[:, t:t + 1],
                            rhs=xt[:, dh * 512:(dh + 1) * 512],
                            start=(t == 0),
                            stop=(t == NT - 1),
                        )

                enc_sb = small_pool.tile([1, BPC * D], F32, tag="enc_sb")
                if stop_after == "mm":
                    nc.scalar.copy(out=enc_sb[:, b * D:b * D + 512], in_=pe0[:])
                    nc.scalar.copy(out=enc_sb[:, b * D + 512:(b + 1) * D],
                                   in_=pe1[:])
                    if b == n_batches - 1:
                        nc.sync.dma_start(out=enc[:], in_=enc_sb[0:1, :])
                    return

                # enc[b] = enc_un / (Q * Z)
                z2 = small_pool.tile([1, 1], F32, tag="z2")
                nc.scalar.mul(out=z2[:], in_=zsum[:], mul=float(Q))
                rz = small_pool.tile([1, 1], F32, tag="rz")
                nc.vector.reciprocal(rz[:], z2[:])
                nc.scalar.activation(
                    out=enc_sb[:, b * D:b * D + 512], in_=pe0[:], func=AFT.Copy,
                    scale=rz[:],
                )
                nc.scalar.activation(
                    out=enc_sb[:, b * D + 512:(b + 1) * D], in_=pe1[:],
                    func=AFT.Copy, scale=rz[:],
                )
                if b == n_batches - 1:
                    nc.sync.dma_start(out=enc[:], in_=enc_sb[0:1, :])

            if reps == 1:
                body()
            elif unroll > 1:
                # unrolled loop body: consecutive reps rotate through the
                # tile pools, letting the next rep's P/x DMAs overlap this
                # rep's batch compute (the plain loop edge serializes).
                if reps // unroll > 0:
                    with tc.For_i(0, reps // unroll, 1):
                        for _ in range(unroll):
                            body()
                for _ in range(reps % unroll):
                    body()
            else:
                with tc.For_i(0, reps, 1):
                    body()

    return nc


def _shard_inputs(embeds_x, embeds_y, P, pshard=False):
    """Build the 8 per-core input maps (host-side layout + bf16 cast)."""
    bf = ml_dtypes.bfloat16
    x = np.asarray(embeds_x, dtype=np.float32)
    y = np.asarray(embeds_y, dtype=np.float32)[:, :, 0]          # [B, CD]
    if pshard:
        ptr_full = P.T.reshape(KT, 128, D).astype(bf)            # [KT, 128, D]
        yk = y.reshape(B, KT, 128).astype(bf)                    # [B, KT, 128]
        in_maps = []
        for c in range(NCORES):
            kt0 = c * KTC
            pt_c = np.zeros((128, KTC * D + PAD), dtype=bf)
            pt_c[:, :KTC * D] = ptr_full[kt0:kt0 + KTC].transpose(
                1, 0, 2).reshape(128, KTC * D)
            ys_c = np.ascontiguousarray(
                yk[:, kt0:kt0 + KTC, :].transpose(2, 1, 0))      # [128, KTC, B]
            sel_c = np.zeros((B, BPC * 128), dtype=bf)
            for b in range(BPC):
                sel_c[c * BPC + b, b * 128:(b + 1) * 128] = 1.0
            sl = slice(c * BPC, (c + 1) * BPC)
            xs_c = np.zeros((128, BPC, NT * D + PAD), dtype=bf)
            xs_c[:, :, :NT * D] = x[sl].reshape(BPC, NT, 128, D).transpose(
                2, 0, 1, 3).reshape(128, BPC, NT * D).astype(bf)
            in_maps.append({"xs": xs_c, "pt": pt_c, "ys": ys_c, "sel": sel_c})
        return in_maps
    # pt[p, k*D + d] = P[d, k*128 + p]
    ptr = np.zeros((128, KT * D + PAD), dtype=bf)
    ptr[:, :KT * D] = P.T.reshape(KT, 128, D).transpose(1, 0, 2).reshape(
        128, KT * D).astype(bf)
    in_maps = []
    for c in range(NCORES):
        sl = slice(c * BPC, (c + 1) * BPC)
        # xs[p, b, t*D + d] = x[b, t*128 + p, d]
        xs_c = np.zeros((128, BPC, NT * D + PAD), dtype=bf)
        xs_c[:, :, :NT * D] = x[sl].reshape(BPC, NT, 128, D).transpose(
            2, 0, 1, 3).reshape(128, BPC, NT * D).astype(bf)
        ys_c = np.ascontiguousarray(
            y[sl].reshape(BPC, KT, 128).transpose(2, 1, 0)
        ).astype(bf)  # [128, KT, BPC]
        in_maps.append({"xs": xs_c, "pt": ptr, "ys": ys_c})
    return in_maps


def kernel(embeds_x, embeds_y, P, M):
    assert int(M) == 2048
    nc = build_nc(reps=1, xhalves=True, fuse_g=4)
    split_sync_waits(nc)  # HW-compile only; CoreSim rejects injected NoOps
    in_maps = _shard_inputs(embeds_x, embeds_y, P)
    res = run_bass_kernel_spmd(nc, in_maps, list(range(NCORES)))
    out = np.concatenate(
        [res.results[c]["enc"].reshape(BPC, D) for c in range(NCORES)], axis=0)
    return out.astype(np.float32)


# revision 6
# speedup vs baseline: 1.3678x; 1.3678x over previous
"""Trainium2 Bass kernel for the attention-based encoder.

Computation (per batch b):
    a      = P @ y[b]                                  # [D]
    logits = x[b] @ a                                  # [M]
    p_un   = exp(logits - 16)                          # fixed shift (cancels)
    Z      = sum(p_un)
    W[t]   = p_un[t-1] + p_un[t] + p_un[t+1] + p_un[t+2]  (zero-padded), W[M-1] = 0
    enc[b] = (W @ x[b]) / (Q * Z)                      # [D]

Design (each point validated by HW ablation / CoreSim timeline analysis):
  * all HBM traffic in bf16 (x 16 MiB + P 10 MiB per core; rel err 3.6e-3
    vs the 2e-2 gate) - halves the DMA floor and runs every matmul at
    bf16 rate instead of multi-pass fp32.
  * host pre-arranges x and P^T so every DMA is one large transfer with
    contiguous 16-80 KiB per partition, and pads the per-partition row
    by 2176 B: the power-of-2 HBM stride otherwise aliases banks
    (187 -> 315 GB/s measured).
  * x loaded as two half-batch tiles (x pool bufs=3, P chunk pool bufs=3:
    with fewer buffers the Tile scheduler reorders x DMAs ahead of the
    last P chunk and delays `a` by ~20 us).
  * logits: per 4-tile group, one fused DVE mul (broadcast a) + one
    strided DVE reduce over the groups' second halves + four ACT
    512-halved accumulates (ACT-1024 accum and DVE-1024 reduce are both
    ~1.3-1.4 us; 512-splitting + fusion balances DVE and ACT).
  * W = 4-tap window of p via 5 banded matmuls per batch over all tiles
    at once (halo via shifted rhs columns of a zero-padded p).
  * a replicated across partitions with a selector matmul; Z via DVE
    reduce + ones-column matmul; single batched output DMA.
  * repeated-execution (reps>1) builds unroll the loop body by 2: the
    plain hardware-loop edge serializes iterations, while a 2-body block
    lets the next rep's P/x DMAs run under this rep's batch compute
    (136 -> 124 us per rep measured; unroll=4 schedules worse).
  * pshard=True phase A (P split over cores + AllReduce) works and is
    equally accurate, but collectives fail inside a reps-loop in this
    runtime, so it is off by default.

Sharding: data-parallel over batch, 4 batches per core on 8 cores.
"""

import numpy as np
import ml_dtypes

import concourse.bass as bass
import concourse.mybir as mybir
from concourse.tile import TileContext
from concourse.bass_utils import run_bass_kernel_spmd

# ---------------------------------------------------------------------------
# This container's walrus supports only ONE sync wait per instruction ("Too
# many sync wait commands" at codegen otherwise), while Tile freely attaches
# several.  Post-pass: hoist excess waits onto injected same-engine NoOps
# placed immediately before the over-subscribed instruction.
# ---------------------------------------------------------------------------

_MAX_WAITS = 1


def split_sync_waits(nc: bass.Bass) -> None:
    uid = 0
    for fn in nc.m.functions:
        for blk in fn.blocks:
            new_insts = []
            for inst in blk.instructions:
                si = inst.sync_info
                waits = list(si.on_wait) if si and si.on_wait else []
                if len(waits) > _MAX_WAITS:
                    for w in waits[:-_MAX_WAITS]:
                        uid += 1
                        ev = mybir.InstEventSemaphore(
                            name=f"{inst.name}_hw{uid}",
                            opcode="EventSemaphore",
                            ins=[],
                            outs=[],
                            sync_info=mybir.SyncInfo(on_wait=[w], on_update=[]),
                        )
                        ev.engine = inst.engine
                        new_insts.append(ev)
                    si.on_wait = waits[-_MAX_WAITS:]
                new_insts.append(inst)
            blk.instructions[:] = new_insts

# ---------------------------------------------------------------------------

B, M, D, CD = 32, 2048, 1024, 5120
Q = 2
NCORES = 8
BPC = B // NCORES          # batches per core
NT = M // 128              # m-tiles per batch
KT = CD // 128             # k-tiles of the P contraction
KCH = 8                    # k-tiles per P DMA chunk
NCH = KT // KCH
KTC = KT // NCORES  # k-tiles per core when P is sharded
PAD = 1088                 # bf16 elems of per-partition-row padding: breaks
                           # the power-of-2 HBM stride aliasing (187 -> 315 GB/s)
HNT = NT // 2              # m-tiles per xb half-load
F32 = mybir.dt.float32
BF16 = mybir.dt.bfloat16
ALU = mybir.AluOpType
AFT = mybir.ActivationFunctionType


def build_nc(reps: int = 1, n_batches: int = BPC, do_phase_b: bool = True,
             stop_after: str = "full", skip: tuple = (),
             logits_mode: str = "split", xhalves: bool = False,
             n_dve: int = 16, n_gps: int = 0,
             accum_dump: str = "inplace", fuse_g: int = 0,
             n_actg: int = 0, pshard: bool = False,
             unroll: int = 2) -> bass.Bass:
    nc = bass.Bass(num_devices=NCORES) if pshard else bass.Bass()
    xs = nc.declare_dram_parameter("xs", [128, BPC, NT * D + PAD], BF16, isOutput=False)
    if pshard:
        # P sharded over cores: each core holds KTC k-tiles of P^T, the y
        # slices matching those k-tiles for ALL B batches, and a selector
        # that picks this core's batch rows out of the allreduced aT.
        pt = nc.declare_dram_parameter("pt", [128, KTC * D + PAD], BF16,
                                       isOutput=False)
        ys = nc.declare_dram_parameter("ys", [128, KTC, B], BF16,
                                       isOutput=False)
        sel = nc.declare_dram_parameter("sel", [B, BPC * 128], BF16,
                                        isOutput=False)
        cc_in = nc.dram_tensor("cc_in", [B, D], F32, addr_space="Local")
        cc_out = nc.dram_tensor("cc_out", [B, D], F32, addr_space="Local")
    else:
        pt = nc.declare_dram_parameter("pt", [128, KT * D + PAD], BF16,
                                       isOutput=False)
        ys = nc.declare_dram_parameter("ys", [128, KT, BPC], BF16,
                                       isOutput=False)
    enc = nc.declare_dram_parameter("enc", [1, BPC * D], F32, isOutput=True)

    with TileContext(nc) as tc:
        with (
            tc.tile_pool(name="const", bufs=1) as const_pool,
            tc.tile_pool(name="ysp", bufs=1) as ys_pool,
            tc.tile_pool(name="ptp", bufs=3) as pt_pool,
            tc.tile_pool(name="xp", bufs=3) as x_pool,
            tc.tile_pool(name="arep", bufs=1) as arep_pool,
            tc.tile_pool(name="small", bufs=1) as small_pool,
            tc.tile_pool(name="tiny", bufs=2) as tiny_pool,
            tc.tile_pool(name="scr", bufs=3) as scr_pool,
            tc.tile_pool(name="ps", bufs=1, space="PSUM") as psum_pool,
            tc.tile_pool(name="pse", bufs=2, space="PSUM") as psum_e_pool,
        ):
            ones_col = const_pool.tile([128, 1], F32)
            nc.vector.memset(ones_col[:], 1.0)
            nshift = const_pool.tile([128, 1], F32)
            nc.vector.memset(nshift[:], -16.0)
            if pshard:
                ys_sb = const_pool.tile([128, KTC, B], BF16)
                nc.sync.dma_start(out=ys_sb[:], in_=ys[:])
                sel_sb = const_pool.tile([B, BPC * 128], BF16)
                nc.sync.dma_start(out=sel_sb[:], in_=sel[:])
            else:
                ys_sb = const_pool.tile([128, KT, BPC], BF16)
                nc.sync.dma_start(out=ys_sb[:], in_=ys[:])

            # banded matrices for the 4-tap sliding-window sum W = S4 @ p:
            # s4[c, f] = 1 iff f - c in {-2, -1, 0, 1}; corner matrices
            # carry the +-2-element inter-tile halo.
            s4 = const_pool.tile([128, 128], F32)
            nc.gpsimd.memset(s4[:], 0.0)
            for base in (1, 0, -1, -2):
                nc.gpsimd.affine_select(
                    out=s4[:], in_=s4[:], compare_op=ALU.not_equal, fill=1.0,
                    base=base, pattern=[[-1, 128]], channel_multiplier=1,
                )
            sprev = const_pool.tile([128, 128], F32)
            nc.gpsimd.memset(sprev[:], 0.0)
            nc.gpsimd.affine_select(
                out=sprev[:], in_=sprev[:], compare_op=ALU.not_equal, fill=1.0,
                base=-127, pattern=[[-1, 128]], channel_multiplier=1,
            )
            snext = const_pool.tile([128, 128], F32)
            nc.gpsimd.memset(snext[:], 0.0)
            for base in (126, 127):
                nc.gpsimd.affine_select(
                    out=snext[:], in_=snext[:], compare_op=ALU.not_equal, fill=1.0,
                    base=base, pattern=[[-1, 128]], channel_multiplier=1,
                )

            # last-tile variant of s4 with column M-1 zeroed (W[M-1] = 0)
            s4last = const_pool.tile([128, 128], F32)
            nc.gpsimd.memset(s4last[:], 0.0)
            for base in (1, 0, -1, -2):
                nc.gpsimd.affine_select(
                    out=s4last[:], in_=s4last[:], compare_op=ALU.not_equal,
                    fill=1.0, base=base, pattern=[[-1, 128]],
                    channel_multiplier=1,
                )
            nc.gpsimd.affine_select(
                out=s4last[:], in_=s4last[:], compare_op=ALU.not_equal,
                fill=0.0, base=-127, pattern=[[1, 128]], channel_multiplier=0,
            )

            # selector rows: selb[b][k, m] = 1 iff k == b (4 partitions);
            # a_rep[b] = selb[b].T @ aT broadcasts batch b's row of aT to
            # all 128 partitions without an SBUF round-trip.
            selb = []
            for b in range(BPC):
                sb = const_pool.tile([BPC, 128], BF16, name=f"selb{b}")
                nc.gpsimd.memset(sb[:], 0.0)
                nc.gpsimd.affine_select(
                    out=sb[:], in_=sb[:], compare_op=ALU.not_equal, fill=1.0,
                    base=-b, pattern=[[0, 128]], channel_multiplier=1,
                )
                selb.append(sb)

            a_rep = [
                arep_pool.tile([128, D], BF16, tag=f"a_rep{b}", name=f"a_rep{b}")
                for b in range(BPC)
            ]

            def body(_=None):
                if "phase_a" in skip:
                    [nc.vector.memset(ar[:], 0.001) for ar in a_rep]
                    return body_b()

                if pshard:
                    return phase_a_sharded()

                # ---- Phase A: aT[b, d] = sum_k y[b, k] * PT[k, d] ----
                pa0 = psum_pool.tile([BPC, 512], F32, tag="pa0")
                pa1 = psum_pool.tile([BPC, 512], F32, tag="pa1")
                for c in range(NCH):
                    ptc = pt_pool.tile([128, KCH * D], BF16, tag="ptc")
                    nc.sync.dma_start(
                        out=ptc[:], in_=pt[:, c * KCH * D:(c + 1) * KCH * D]
                    )
                    for u in range(KCH):
                        t = c * KCH + u
                        nc.tensor.matmul(
                            pa0[:], lhsT=ys_sb[:, t, :],
                            rhs=ptc[:, u * D:u * D + 512],
                            start=(t == 0), stop=(t == KT - 1),
                        )
                        nc.tensor.matmul(
                            pa1[:], lhsT=ys_sb[:, t, :],
                            rhs=ptc[:, u * D + 512:(u + 1) * D],
                            start=(t == 0), stop=(t == KT - 1),
                        )
                aT_sb = small_pool.tile([BPC, D], BF16, tag="aT")
                nc.vector.tensor_copy(aT_sb[:, 0:512], pa0[:])
                nc.vector.tensor_copy(aT_sb[:, 512:1024], pa1[:])

                # replicate a[b] across all 128 partitions (selector matmul)
                for b in range(BPC):
                    for dh in range(2):
                        pr = psum_pool.tile([128, 512], F32, tag="pr")
                        nc.tensor.matmul(
                            pr[:], lhsT=selb[b][:],
                            rhs=aT_sb[:, dh * 512:(dh + 1) * 512],
                            start=True, stop=True,
                        )
                        nc.vector.tensor_copy(
                            a_rep[b][:, dh * 512:(dh + 1) * 512], pr[:]
                        )

                if not do_phase_b:
                    for b in range(BPC):
                        nc.gpsimd.dma_start(
                            out=enc[0, b * D:b * D + 512],
                            in_=a_rep[b][0:1, 0:512],
                        )
                    return
                return body_b()

            def phase_a_sharded():
                # partial aT for ALL batches from this core's P k-tiles,
                # then an 8-core AllReduce combines the partials.
                pa0 = psum_pool.tile([B, 512], F32, tag="pa0")
                pa1 = psum_pool.tile([B, 512], F32, tag="pa1")
                ptc = pt_pool.tile([128, KTC * D], BF16, tag="ptc")
                nc.sync.dma_start(out=ptc[:], in_=pt[:, 0:KTC * D])
                for kt in range(KTC):
                    nc.tensor.matmul(
                        pa0[:], lhsT=ys_sb[:, kt, :],
                        rhs=ptc[:, kt * D:kt * D + 512],
                        start=(kt == 0), stop=(kt == KTC - 1),
                    )
                    nc.tensor.matmul(
                        pa1[:], lhsT=ys_sb[:, kt, :],
                        rhs=ptc[:, kt * D + 512:(kt + 1) * D],
                        start=(kt == 0), stop=(kt == KTC - 1),
                    )
                aT_part = small_pool.tile([B, D], F32, tag="aT_part")
                nc.vector.tensor_copy(aT_part[:, 0:512], pa0[:])
                nc.vector.tensor_copy(aT_part[:, 512:1024], pa1[:])
                nc.sync.dma_start(out=cc_in[:], in_=aT_part[:])
                nc.gpsimd.collective_compute(
                    "AllReduce", ALU.add,
                    replica_groups=[list(range(NCORES))],
                    ins=[cc_in[:]], outs=[cc_out[:]],
                )
                aT_f32 = small_pool.tile([B, D], F32, tag="aT_f32")
                nc.sync.dma_start(out=aT_f32[:], in_=cc_out[:])
                aT_sb = small_pool.tile([B, D], BF16, tag="aT")
                nc.vector.tensor_copy(aT_sb[:], aT_f32[:])

                for b in range(BPC):
                    for dh in range(2):
                        pr = psum_pool.tile([128, 512], F32, tag="pr")
                        nc.tensor.matmul(
                            pr[:], lhsT=sel_sb[:, b * 128:(b + 1) * 128],
                            rhs=aT_sb[:, dh * 512:(dh + 1) * 512],
                            start=True, stop=True,
                        )
                        nc.vector.tensor_copy(
                            a_rep[b][:, dh * 512:(dh + 1) * 512], pr[:]
                        )
                if not do_phase_b:
                    for b in range(BPC):
                        nc.gpsimd.dma_start(
                            out=enc[0, b * D:b * D + 512],
                            in_=a_rep[b][0:1, 0:512],
                        )
                    return
                return body_b()

            def body_b():
                # ---- Phase B: per-batch attention ----
                # n_dve: reduce-halves handed to DVE tensor_reduce instead of
                # ACT accumulate; n_gps: muls offloaded to GpSimd.
                for b in range(n_batches):
                    if xhalves:
                        # two half-batch loads so compute starts ~6.5us earlier
                        xh = []
                        for h in range(2):
                            xt_ = x_pool.tile([128, HNT * D], BF16, tag=f"xh{h}")
                            nc.sync.dma_start(
                                out=xt_[:],
                                in_=xs[:, b, h * HNT * D:(h + 1) * HNT * D],
                            )
                            xh.append(xt_)
                    else:
                        xb = x_pool.tile([128, NT * D], BF16, tag="xh0")
                        nc.sync.dma_start(out=xb[:], in_=xs[:, b, 0:NT * D])
                        xh = [xb[:, 0:HNT * D], xb[:, HNT * D:NT * D]]

                    # logits[m] = x[m, :] . a  - DVE mul per m-tile, then the
                    # free-dim reduce as two 512-halves on ACT (420ns each vs
                    # 1439ns for a 1024 accum / 1266ns for a DVE reduce).
                    logits_a = tiny_pool.tile([128, NT], F32, tag="logits_a")
                    logits_b = tiny_pool.tile([128, NT], F32, tag="logits_b")
                    if "logits" in skip:
                        nc.vector.memset(logits_a[:], 0.005)
                        nc.vector.memset(logits_b[:], 0.005)
                    elif fuse_g:
                        # G-tile fused DVE ops: one mul + one strided reduce
                        # per group amortizes the ~160-cycle DVE op overhead.
                        G = fuse_g
                        for g in range(NT // G):
                            t0 = g * G
                            h = t0 // HNT
                            xt = xh[h][:, (t0 % HNT) * D:(t0 % HNT + G) * D]
                            scratch = scr_pool.tile([128, G * D], BF16,
                                                    tag="scratch")
                            nc.vector.tensor_mul(
                                scratch[:].rearrange("p (g d) -> p g d", g=G),
                                xt.rearrange("p (g d) -> p g d", g=G),
                                a_rep[b][:, None, :].broadcast_to([128, G, D]),
                            )
                            if g < n_actg:
                                # ACT takes this group's second halves too
                                for u in range(G):
                                    nc.scalar.activation(
                                        out=scratch[:, u * D + 512:(u + 1) * D],
                                        in_=scratch[:, u * D + 512:(u + 1) * D],
                                        func=AFT.Copy,
                                        accum_out=logits_b[:, t0 + u:t0 + u + 1],
                                    )
                            else:
                                sv = scratch[:].rearrange(
                                    "p (g two h) -> p g two h", g=G, two=2)
                                nc.vector.tensor_reduce(
                                    out=logits_b[:, t0:t0 + G],
                                    in_=sv[:, :, 1, :],
                                    axis=mybir.AxisListType.X, op=ALU.add,
                                )
                            for u in range(G):
                                nc.scalar.activation(
                                    out=scratch[:, u * D:u * D + 512],
                                    in_=scratch[:, u * D:u * D + 512],
                                    func=AFT.Copy,
                                    accum_out=logits_a[:, t0 + u:t0 + u + 1],
                                )
                    else:
                        for t in range(NT):
                            xt = xh[t // HNT][:, (t % HNT) * D:(t % HNT + 1) * D]
                            scratch = scr_pool.tile([128, D], BF16, tag="scratch")
                            mul_eng = nc.gpsimd if t < n_gps else nc.vector
                            mul_eng.tensor_mul(scratch[:], xt, a_rep[b][:])
                            if t < n_dve - NT:
                                nc.vector.tensor_reduce(
                                    out=logits_a[:, t:t + 1],
                                    in_=scratch[:, 0:512],
                                    axis=mybir.AxisListType.X, op=ALU.add,
                                )
                            else:
                                if accum_dump == "psum":
                                    dmp = psum_pool.tile([128, 512], F32,
                                                         tag="pa0")
                                    outa = dmp[:]
                                elif accum_dump == "sbuf":
                                    dmp = scr_pool.tile([128, 512], BF16,
                                                        tag="dump")
                                    outa = dmp[:]
                                else:
                                    outa = scratch[:, 0:512]
                                nc.scalar.activation(
                                    out=outa, in_=scratch[:, 0:512],
                                    func=AFT.Copy, accum_out=logits_a[:, t:t + 1],
                                )
                            if t < n_dve:
                                nc.vector.tensor_reduce(
                                    out=logits_b[:, t:t + 1],
                                    in_=scratch[:, 512:1024],
                                    axis=mybir.AxisListType.X, op=ALU.add,
                                )
                            else:
                                if accum_dump == "psum":
                                    dmp = psum_pool.tile([128, 512], F32,
                                                         tag="pa1")
                                    outb = dmp[:]
                                elif accum_dump == "sbuf":
                                    dmp = scr_pool.tile([128, 512], BF16,
                                                        tag="dump")
                                    outb = dmp[:]
                                else:
                                    outb = scratch[:, 512:1024]
                                nc.scalar.activation(
                                    out=outb, in_=scratch[:, 512:1024],
                                    func=AFT.Copy,
                                    accum_out=logits_b[:, t:t + 1],
                                )
                    nc.vector.tensor_add(logits_a[:], logits_a[:], logits_b[:])

                    if stop_after == "logits":
                        nc.sync.dma_start(out=enc[0, b * D:b * D + NT], in_=logits_a[0:1, :])
                        continue

                    if "softmax" in skip:
                        zsum = tiny_pool.tile([1, 1], F32, tag="zsum")
                        nc.vector.memset(zsum[:], 1.0)
                        w_pm = tiny_pool.tile([128, NT], BF16, tag="w_pm")
                        nc.vector.memset(w_pm[:], 0.01)
                        do_tail(b, xh, w_pm, zsum)
                        continue

                    # softmax without the row max: fixed shift (cancels in
                    # enc = sum(W x)/(Q Z)); exp on ACT in [128, NT] space.
                    p_pad = tiny_pool.tile([128, NT + 2], F32, tag="p_pad")
                    zcol = tiny_pool.tile([128, 1], F32, tag="zcol")
                    nc.vector.memset(p_pad[:, 0:1], 0.0)
                    nc.vector.memset(p_pad[:, NT + 1:NT + 2], 0.0)
                    nc.scalar.activation(
                        out=p_pad[:, 1:NT + 1],
                        in_=logits_a[:],
                        func=AFT.Exp,
                        bias=nshift[:],
                        scale=1.0,
                    )
                    nc.vector.tensor_reduce(
                        out=zcol[:], in_=p_pad[:, 1:NT + 1],
                        axis=mybir.AxisListType.X, op=ALU.add,
                    )

                    # Z = sum over partitions of zcol (ones-column matmul)
                    z_ps = psum_pool.tile([1, 1], F32, tag="pr")
                    nc.tensor.matmul(z_ps[:], lhsT=zcol[:], rhs=ones_col[:],
                                     start=True, stop=True)
                    zsum = tiny_pool.tile([1, 1], F32, tag="zsum")
                    nc.scalar.copy(out=zsum[:], in_=z_ps[:])

                    # W = 4-tap window of p: banded matmuls over tiles
                    # 0..14 at once (halo via shifted rhs columns of p_pad);
                    # the last tile separately with s4last (W[M-1] = 0).
                    w_ps = psum_pool.tile([128, NT], F32, tag="w_ps")
                    nc.tensor.matmul(w_ps[:, 0:NT - 1], lhsT=s4[:],
                                     rhs=p_pad[:, 1:NT], start=True, stop=False)
                    nc.tensor.matmul(w_ps[:, 0:NT - 1], lhsT=sprev[:],
                                     rhs=p_pad[:, 0:NT - 1], start=False,
                                     stop=False)
                    nc.tensor.matmul(w_ps[:, 0:NT - 1], lhsT=snext[:],
                                     rhs=p_pad[:, 2:NT + 1], start=False,
                                     stop=True)
                    nc.tensor.matmul(w_ps[:, NT - 1:NT], lhsT=s4last[:],
                                     rhs=p_pad[:, NT:NT + 1], start=True,
                                     stop=False)
                    nc.tensor.matmul(w_ps[:, NT - 1:NT], lhsT=sprev[:],
                                     rhs=p_pad[:, NT - 1:NT], start=False,
                                     stop=True)
                    w_pm = tiny_pool.tile([128, NT], BF16, tag="w_pm")
                    nc.scalar.copy(out=w_pm[:], in_=w_ps[:])

                    do_tail(b, xh, w_pm, zsum)

            def do_tail(b, xh, w_pm, zsum):
                # enc_un[d] = sum_m W[m] x[m, d]   (PE, W cols as weights)
                pe0 = psum_e_pool.tile([1, 512], F32, tag="pe0")
                pe1 = psum_e_pool.tile([1, 512], F32, tag="pe1")
                for t in range(NT):
                    xt = xh[t // HNT][:, (t % HNT) * D:(t % HNT + 1) * D]
                    for dh, pe in enumerate((pe0, pe1)):
                        nc.tensor.matmul(
                            pe[:],
                            lhsT=w_pm[:, t:t + 1],
                            rhs=xt[:, dh * 512:(dh + 1) * 512],
                            start=(t == 0),
                            stop=(t == NT - 1),
                        )

                enc_sb = small_pool.tile([1, BPC * D], F32, tag="enc_sb")
                if stop_after == "mm":
                    nc.scalar.copy(out=enc_sb[:, b * D:b * D + 512], in_=pe0[:])
                    nc.scalar.copy(out=enc_sb[:, b * D + 512:(b + 1) * D],
                                   in_=pe1[:])
                    if b == n_batches - 1:
                        nc.sync.dma_start(out=enc[:], in_=enc_sb[0:1, :])
                    return

                # enc[b] = enc_un / (Q * Z)
                z2 = small_pool.tile([1, 1], F32, tag="z2")
                nc.scalar.mul(out=z2[:], in_=zsum[:], mul=float(Q))
                rz = small_pool.tile([1, 1], F32, tag="rz")
                nc.vector.reciprocal(rz[:], z2[:])
                nc.scalar.activation(
                    out=enc_sb[:, b * D:b * D + 512], in_=pe0[:], func=AFT.Copy,
                    scale=rz[:],
                )
                nc.scalar.activation(
                    out=enc_sb[:, b * D + 512:(b + 1) * D], in_=pe1[:],
                    func=AFT.Copy, scale=rz[:],
                )
                if b == n_batches - 1:
                    nc.sync.dma_start(out=enc[:], in_=enc_sb[0:1, :])

            if reps == 1:
                body()
            elif unroll > 1:
                # unrolled loop body: consecutive reps rotate through the
                # tile pools, letting the next rep's P/x DMAs overlap this
                # rep's batch compute (the plain loop edge serializes; a
                # non-loop epilogue body also kills the pipelining, so the
                # rep count rounds UP to a multiple of unroll - reps are
                # idempotent, an extra one only costs time).
                with tc.For_i(0, (reps + unroll - 1) // unroll, 1):
                    for _ in range(unroll):
                        body()
            else:
                with tc.For_i(0, reps, 1):
                    body()

    return nc


def _shard_inputs(embeds_x, embeds_y, P, pshard=False):
    """Build the 8 per-core input maps (host-side layout + bf16 cast)."""
    bf = ml_dtypes.bfloat16
    x = np.asarray(embeds_x, dtype=np.float32)
    y = np.asarray(embeds_y, dtype=np.float32)[:, :, 0]          # [B, CD]
    if pshard:
        ptr_full = P.T.reshape(KT, 128, D).astype(bf)            # [KT, 128, D]
        yk = y.reshape(B, KT, 128).astype(bf)                    # [B, KT, 128]
        in_maps = []
        for c in range(NCORES):
            kt0 = c * KTC
            pt_c = np.zeros((128, KTC * D + PAD), dtype=bf)
            pt_c[:, :KTC * D] = ptr_full[kt0:kt0 + KTC].transpose(
                1, 0, 2).reshape(128, KTC * D)
            ys_c = np.ascontiguousarray(
                yk[:, kt0:kt0 + KTC, :].transpose(2, 1, 0))      # [128, KTC, B]
            sel_c = np.zeros((B, BPC * 128), dtype=bf)
            for b in range(BPC):
                sel_c[c * BPC + b, b * 128:(b + 1) * 128] = 1.0
            sl = slice(c * BPC, (c + 1) * BPC)
            xs_c = np.zeros((128, BPC, NT * D + PAD), dtype=bf)
            xs_c[:, :, :NT * D] = x[sl].reshape(BPC, NT, 128, D).transpose(
                2, 0, 1, 3).reshape(128, BPC, NT * D).astype(bf)
            in_maps.append({"xs": xs_c, "pt": pt_c, "ys": ys_c, "sel": sel_c})
        return in_maps
    # pt[p, k*D + d] = P[d, k*128 + p]
    ptr = np.zeros((128, KT * D + PAD), dtype=bf)
    ptr[:, :KT * D] = P.T.reshape(KT, 128, D).transpose(1, 0, 2).reshape(
        128, KT * D).astype(bf)
    in_maps = []
    for c in range(NCORES):
        sl = slice(c * BPC, (c + 1) * BPC)
        # xs[p, b, t*D + d] = x[b, t*128 + p, d]
        xs_c = np.zeros((128, BPC, NT * D + PAD), dtype=bf)
        xs_c[:, :, :NT * D] = x[sl].reshape(BPC, NT, 128, D).transpose(
            2, 0, 1, 3).reshape(128, BPC, NT * D).astype(bf)
        ys_c = np.ascontiguousarray(
            y[sl].reshape(BPC, KT, 128).transpose(2, 1, 0)
        ).astype(bf)  # [128, KT, BPC]
        in_maps.append({"xs": xs_c, "pt": ptr, "ys": ys_c})
    return in_maps


def kernel(embeds_x, embeds_y, P, M):
    assert int(M) == 2048
    nc = build_nc(reps=1, xhalves=True, fuse_g=4)
    split_sync_waits(nc)  # HW-compile only; CoreSim rejects injected NoOps
    in_maps = _shard_inputs(embeds_x, embeds_y, P)
    res = run_bass_kernel_spmd(nc, in_maps, list(range(NCORES)))
    out = np.concatenate(
        [res.results[c]["enc"].reshape(BPC, D) for c in range(NCORES)], axis=0)
    return out.astype(np.float32)
